# revision 39
# baseline (speedup 1.0000x reference)
"""Trainium2 Bass kernel for nn_MixtureOfExpertsLayer (moe_routing).

Sparse dispatch: top-2 routing is computed on the host (the router is a
tiny [8192,1024]@[1024,4] GEMM); tokens are gathered per expert and
sharded across the 8 cores so each core runs a fixed 512-token slab
through each of the 4 experts — half the dense FLOPs.  The linear
chains inside experts 1/2 are pre-folded on the host
(eq_w@wv@wo and syn_w@(I+wv@wo)), removing another ~11% of matmul work.

Device compute is bf16 (PSUM accumulates fp32).  Weights are pre-packed
on the host into the exact [p, kc, 256-col] tile layout the tensor
engine wants, so every DMA is a fully contiguous 0.5-2MB block.
Activations stay feature-major [128p, chunk, tok]; every matmul has a
512-token moving dim (full PE rate).  Expert outputs come back
feature-major [H, 512] fp32; the host applies the top-2 softmax gates
and scatter-adds into the final output.  Tokens beyond the
4096-per-expert device capacity (a few dozen when routing is balanced)
are computed on the host in fp64.
"""
import math

import numpy as np
import ml_dtypes

import concourse.bass as bass
import concourse.mybir as mybir
import concourse.tile as tile
from concourse import bacc
from concourse.alu_op_type import AluOpType
from concourse.bass_utils import run_bass_kernel_spmd

F32 = mybir.dt.float32
BF16 = mybir.dt.bfloat16
ACT = mybir.ActivationFunctionType
AX = mybir.AxisListType
OP = AluOpType
BF = ml_dtypes.bfloat16

N_CORES = 8
B, S, H, I, E = 4, 2048, 1024, 4096, 4
P = 128
T = 512                   # tokens per expert per core
CAP = N_CORES * T         # device capacity per expert
KC = H // P               # 8
KI = I // P               # 32
K2 = (2 * H) // P         # 16

# packed weight dram tensors: name -> (n_256col_blocks, contraction_chunks)
PACKED_W = {
    "w1p": (I // 256, KC), "w3p": (I // 256, KC), "m1p": (I // 256, KC),
    "w2p": (H // 256, KI), "m2p": (H // 256, KI),
    "c1p": (2 * H // 256, KC), "f1p": (2 * H // 256, KC),
    "c2p": (H // 256, K2), "f2p": (H // 256, K2),
    "a2p": (H // 256, KC), "genp": (H // 256, KC),
}
# biases live in one packed [P, sum] f32 tensor; name -> n_chunks
BIASES = {
    "c1b": K2, "c2b": KC,
    "a2b": KC, "f1b": K2, "f2b": KC,
    "n1g": KC, "n1b": KC, "n2g": KC, "n2b": KC, "genb": KC,
    "m1b": KI, "m2b": KC, "zb": KC,
}
BIAS_OFF = {}
_off = 0
for _n, _c in BIASES.items():
    BIAS_OFF[_n] = _off
    _off += _c
BIAS_COLS = _off


def build_moe_sparse():
    nc = bacc.Bacc("TRN2", target_bir_lowering=False, debug=False)

    xg = [nc.dram_tensor(f"xg{e}", [P, KC, T], BF16, kind="ExternalInput")
          for e in range(E)]
    wd = {n: nc.dram_tensor(n, [nb, P, kcc, 256], BF16, kind="ExternalInput")
          for n, (nb, kcc) in PACKED_W.items()}
    ball = nc.dram_tensor("ball", [P, BIAS_COLS], F32, kind="ExternalInput")
    ys = [nc.dram_tensor(f"y{e}", [P, KC, T], F32, kind="ExternalOutput")
          for e in range(E)]

    with tile.TileContext(nc) as tc:
        with (
            tc.tile_pool(name="const", bufs=1) as cpool,
            tc.tile_pool(name="xg", bufs=2) as xpool,
            tc.tile_pool(name="h1", bufs=1) as hpool,
            tc.tile_pool(name="inter", bufs=1) as ipool,
            tc.tile_pool(name="ws", bufs=5) as wsp,     # KC-contraction blocks
            tc.tile_pool(name="ws2", bufs=3) as wsp2,   # K2-contraction blocks
            tc.tile_pool(name="wb", bufs=2) as wbp,     # KI-contraction blocks
            tc.tile_pool(name="yev", bufs=3) as ypool,  # output eviction
            tc.tile_pool(name="lns", bufs=1) as lnsp,
            tc.tile_pool(name="lnt", bufs=2) as lntp,
            tc.tile_pool(name="sq", bufs=2) as sqp,
            tc.tile_pool(name="ps", bufs=4, space=bass.MemorySpace.PSUM) as psp,
            tc.tile_pool(name="pss", bufs=2, space=bass.MemorySpace.PSUM) as pssp,
            tc.tile_pool(name="psb", bufs=2, space=bass.MemorySpace.PSUM) as psbp,
        ):
            # ---- constants ------------------------------------------------
            ones_cf = cpool.tile([P, 1], F32, tag="ones_cf")
            nc.vector.memset(ones_cf[:], 1.0)
            ones_c = cpool.tile([P, 1], BF16, tag="ones_c")
            nc.vector.tensor_copy(ones_c[:], ones_cf[:])
            ones_rf = cpool.tile([1, P], F32, tag="ones_rf")
            nc.vector.memset(ones_rf[:], 1.0)
            ones_r = cpool.tile([1, P], BF16, tag="ones_r")
            nc.vector.tensor_copy(ones_r[:], ones_rf[:])

            bt_all = cpool.tile([P, BIAS_COLS], F32, tag="ball")
            bt = {n: bt_all[:, BIAS_OFF[n]:BIAS_OFF[n] + nch]
                  for n, nch in BIASES.items()}

            def load_xg(e, split=1):
                t_ = xpool.tile([P, KC, T], BF16, tag="xg", name=f"xgt{e}")
                step = KC // split
                for i in range(split):
                    sl = slice(i * step, (i + 1) * step)
                    nc.sync.dma_start(t_[:, sl, :], xg[e].ap()[:, sl, :])
                return t_

            h1 = hpool.tile([P, KI, T], BF16, tag="h1")

            # ---- helpers --------------------------------------------------
            def up_proj(dst, wname, src, src_kc, act, bias, blocks=None,
                        pool=None):
                """dst[:, c, :] = act(Wc.T @ src + bias_c), streamed in
                256-col blocks.  dst chunk c = 2*b + ml."""
                pool = pool or wsp
                nb = PACKED_W[wname][0]
                for b_ in (range(nb) if blocks is None else blocks):
                    wc = pool.tile([P, src_kc, 256], BF16, tag="w")
                    nc.sync.dma_start(wc[:], wd[wname].ap()[b_])
                    for ml in range(2):
                        c = 2 * b_ + ml
                        ps = psp.tile([P, T], F32, tag="mm")
                        for kc in range(src_kc):
                            nc.tensor.matmul(
                                ps[:], wc[:, kc, ml * P:(ml + 1) * P],
                                src[:, kc, :],
                                start=(kc == 0), stop=(kc == src_kc - 1))
                        b_sl = None if bias is None else bias[:, c:c + 1]
                        nc.scalar.activation(dst[:, c, :], ps[:], act,
                                             bias=b_sl)

            def out_proj(ydram, wname, src, src_kc, bias, wpool):
                """y[:, c, :] = Wc.T @ src + bias_c -> DMA to DRAM (fp32)."""
                nb = PACKED_W[wname][0]
                for b_ in range(nb):
                    wc = wpool.tile([P, src_kc, 256], BF16, tag="w")
                    nc.sync.dma_start(wc[:], wd[wname].ap()[b_])
                    for ml in range(2):
                        c = 2 * b_ + ml
                        ps = psp.tile([P, T], F32, tag="mm")
                        for kc in range(src_kc):
                            nc.tensor.matmul(
                                ps[:], wc[:, kc, ml * P:(ml + 1) * P],
                                src[:, kc, :],
                                start=(kc == 0), stop=(kc == src_kc - 1))
                        yt = ypool.tile([P, T], F32, tag="y")
                        nc.vector.tensor_scalar(yt[:], ps[:],
                                                bias[:, c:c + 1], None, OP.add)
                        nc.sync.dma_start(ydram.ap()[:, c, :], yt[:])

            def ln_stats(src, tag):
                """Mean/rstd rows (bf16 [1,T]) of feature-major src."""
                ssum = pssp.tile([1, T], F32, tag="st")
                for kc in range(KC):
                    nc.tensor.matmul(ssum[:], ones_c[:], src[:, kc, :],
                                     start=(kc == 0), stop=(kc == KC - 1))
                ssq = pssp.tile([1, T], F32, tag="st")
                for kc in range(KC):
                    sqc = sqp.tile([P, T], BF16, tag="sq")
                    nc.vector.tensor_tensor(sqc[:], src[:, kc, :],
                                            src[:, kc, :], OP.mult)
                    nc.tensor.matmul(ssq[:], ones_c[:], sqc[:],
                                     start=(kc == 0), stop=(kc == KC - 1))
                mu = lnsp.tile([1, T], F32, tag="mu")
                nc.vector.tensor_scalar(mu[:], ssum[:], 1.0 / H, None, OP.mult)
                msq = lnsp.tile([1, T], F32, tag="ms")
                nc.vector.tensor_scalar(msq[:], ssq[:], 1.0 / H, None, OP.mult)
                mu_b = lnsp.tile([1, T], BF16, tag=tag + "mb")
                nc.vector.tensor_copy(mu_b[:], mu[:])
                mu2 = lnsp.tile([1, T], F32, tag="m2")
                nc.vector.tensor_tensor(mu2[:], mu[:], mu[:], OP.mult)
                var = lnsp.tile([1, T], F32, tag="va")
                nc.vector.scalar_tensor_tensor(var[:], msq[:], 1e-5, mu2[:],
                                               OP.add, OP.subtract)
                sdev = lnsp.tile([1, T], F32, tag="sd")
                nc.scalar.activation(sdev[:], var[:], ACT.Sqrt)
                rstd_f = lnsp.tile([1, T], F32, tag="rf")
                nc.vector.reciprocal(rstd_f[:], sdev[:])
                rs_b = lnsp.tile([1, T], BF16, tag=tag + "rb")
                nc.vector.tensor_copy(rs_b[:], rstd_f[:])
                return mu_b, rs_b

            def ln_bcast(mu_b, rs_b):
                """Broadcast [1,T] mean/rstd rows to [P,T] via K=1 matmul."""
                mub = psbp.tile([P, T], F32, tag="bc")
                nc.tensor.matmul(mub[:], ones_r[:], mu_b[:], start=True,
                                 stop=True)
                rsb = psbp.tile([P, T], F32, tag="bc")
                nc.tensor.matmul(rsb[:], ones_r[:], rs_b[:], start=True,
                                 stop=True)
                return mub, rsb

            def ln_norm(dst, src, mub, rsb, g_t, b_t):
                """dst = (src - mu) * rstd * g + b  (bf16 out, DVE only)."""
                for kc in range(KC):
                    t1_ = lntp.tile([P, T], F32, tag="lnt")
                    nc.vector.tensor_tensor(t1_[:], src[:, kc, :], mub[:],
                                            OP.subtract)
                    nc.vector.tensor_tensor(t1_[:], t1_[:], rsb[:], OP.mult)
                    nc.vector.tensor_scalar(dst[:, kc, :], t1_[:],
                                            g_t[:, kc:kc + 1],
                                            b_t[:, kc:kc + 1],
                                            OP.mult, OP.add)

            def ln_apply(dst, src, mu_b, rs_b, g_t, b_t):
                mub, rsb = ln_bcast(mu_b, rs_b)
                ln_norm(dst, src, mub, rsb, g_t, b_t)

            # ---- expert 0: SwiGLU ----------------------------------------
            xt0 = load_xg(0, split=2)
            for b_ in range(I // 256):
                wa = wsp.tile([P, KC, 256], BF16, tag="w")
                nc.sync.dma_start(wa[:], wd["w1p"].ap()[b_])
                wb = wsp.tile([P, KC, 256], BF16, tag="w")
                nc.sync.dma_start(wb[:], wd["w3p"].ap()[b_])
                # defer non-critical loads so startup DMA bandwidth goes
                # to xg0 + the first SwiGLU weight blocks
                if b_ == 4:
                    xt2 = load_xg(2)
                if b_ == 6:
                    nc.sync.dma_start(bt_all[:], ball.ap())
                for ml in range(2):
                    c = 2 * b_ + ml
                    psa = psp.tile([P, T], F32, tag="mm")
                    psb = psp.tile([P, T], F32, tag="mm")
                    for kc in range(KC):
                        nc.tensor.matmul(psa[:], wa[:, kc, ml * P:(ml + 1) * P],
                                         xt0[:, kc, :],
                                         start=(kc == 0), stop=(kc == KC - 1))
                    for kc in range(KC):
                        nc.tensor.matmul(psb[:], wb[:, kc, ml * P:(ml + 1) * P],
                                         xt0[:, kc, :],
                                         start=(kc == 0), stop=(kc == KC - 1))
                    sa = ypool.tile([P, T], F32, tag="sa")
                    nc.scalar.activation(sa[:], psa[:], ACT.Silu)
                    nc.vector.tensor_tensor(h1[:, c, :], psb[:], sa[:],
                                            OP.mult)
            out_proj(ys[0], "w2p", h1, KI, bt["zb"], wbp)

            # ---- expert 2 (part 1): folded front + LN1 stats -------------
            t2 = ipool.tile([P, KC, T], BF16, tag="tA", name="t2")
            up_proj(t2, "a2p", xt2, KC, ACT.Identity, bt["a2b"])
            mu1, rs1 = ln_stats(t2, "l1")

            # ---- expert 1 (filler for LN1 latency); c1p holds A1@C1 ------
            xt1 = load_xg(1)
            g1 = ipool.tile([P, K2, T], BF16, tag="tD", name="g1")
            up_proj(g1, "c1p", xt1, KC, ACT.Gelu, bt["c1b"])
            h2 = ipool.tile([P, KC, T], BF16, tag="tC", name="h2")
            ln_apply(h2, t2, mu1, rs1, bt["n1g"], bt["n1b"])
            out_proj(ys[1], "c2p", g1, K2, bt["c2b"], wsp2)

            # ---- expert 2 (part 2): FF + residual + LN2 stats ------------
            g2 = ipool.tile([P, K2, T], BF16, tag="tD", name="g2")
            up_proj(g2, "f1p", h2, KC, ACT.Relu, bt["f1b"])
            ffa = ipool.tile([P, KC, T], BF16, tag="tB", name="ffa")
            nb_f2 = PACKED_W["f2p"][0]
            for b_ in range(nb_f2):
                wc = wsp2.tile([P, K2, 256], BF16, tag="w")
                nc.sync.dma_start(wc[:], wd["f2p"].ap()[b_])
                for ml in range(2):
                    c = 2 * b_ + ml
                    ps = psp.tile([P, T], F32, tag="mm")
                    for kc in range(K2):
                        nc.tensor.matmul(ps[:], wc[:, kc, ml * P:(ml + 1) * P],
                                         g2[:, kc, :],
                                         start=(kc == 0), stop=(kc == K2 - 1))
                    # ffa = ff + f2b + h2   (residual)
                    nc.vector.scalar_tensor_tensor(
                        ffa[:, c, :], ps[:], bt["f2b"][:, c:c + 1],
                        h2[:, c, :], OP.add, OP.add)
            # ---- expert 3 up-proj interleaved with LN2 + generator -------
            # E3 blocks fill the PE while the DVE drains ffa evictions,
            # computes LN2 stats rows, and normalizes h2b.
            xt3 = load_xg(3)
            up_proj(h1, "m1p", xt3, KC, ACT.Gelu, bt["m1b"],
                    blocks=range(0, 2))
            mu2, rs2 = ln_stats(ffa, "l2")
            up_proj(h1, "m1p", xt3, KC, ACT.Gelu, bt["m1b"],
                    blocks=range(2, 8))
            mub2, rsb2 = ln_bcast(mu2, rs2)
            up_proj(h1, "m1p", xt3, KC, ACT.Gelu, bt["m1b"],
                    blocks=range(8, 16))
            h2b = ipool.tile([P, KC, T], BF16, tag="tA", name="h2b")
            ln_norm(h2b, ffa, mub2, rsb2, bt["n2g"], bt["n2b"])
            out_proj(ys[2], "genp", h2b, KC, bt["genb"], wsp)

            # ---- expert 3 down-projection --------------------------------
            out_proj(ys[3], "m2p", h1, KI, bt["m2b"], wbp)

    nc.compile()
    return nc


_PROGRAM = None


def _get_program():
    global _PROGRAM
    if _PROGRAM is None:
        _PROGRAM = build_moe_sparse()
    return _PROGRAM


def run_cores(nc, in_maps, trace=False, trace_cores=None):
    if trace:
        _install_ntff_shim()
    return run_bass_kernel_spmd(nc, in_maps, core_ids=list(range(len(in_maps))),
                                trace=trace, trace_cores=trace_cores)


# ---- host side ---------------------------------------------------------
def _gelu(x):
    try:
        from scipy.special import erf
        return 0.5 * x * (1.0 + erf(x / math.sqrt(2.0)))
    except ImportError:
        ve = np.vectorize(math.erf)
        return 0.5 * x * (1.0 + ve(x / math.sqrt(2.0)))


def _ln64(h, g, b, eps=1e-5):
    mu = h.mean(-1, keepdims=True)
    var = ((h - mu) ** 2).mean(-1, keepdims=True)
    return (h - mu) / np.sqrt(var + eps) * g + b


def _pack_w(w, kcc):
    """[K, M] fp64 -> [M//256, P, kcc, 256] bf16 contiguous tile blocks."""
    K, M = w.shape
    assert K == kcc * P
    r = w.reshape(kcc, P, M)
    blocks = [np.ascontiguousarray(r[:, :, b * 256:(b + 1) * 256]
                                   .transpose(1, 0, 2))
              for b in range(M // 256)]
    return np.stack(blocks, 0).astype(BF)


def _pack_b(b):
    n = b.shape[0] // P
    return np.ascontiguousarray(b.reshape(n, P).T.astype(np.float32))


def prepare(inputs):
    f64 = lambda n: np.asarray(inputs[n], np.float64)
    x = np.asarray(inputs["x"], np.float32).reshape(-1, H)

    # routing (host, fp64)
    lg = x.astype(np.float64) @ f64("router_w")
    lg += f64("router_b") + f64("load_balancer")
    sel = np.argsort(-lg, axis=1, kind="stable")[:, :2]
    ls = np.take_along_axis(lg, sel, 1)
    ew = np.exp(ls - ls.max(1, keepdims=True))
    gates = ew / ew.sum(1, keepdims=True)

    # folded weights (fp64)
    F = {}
    F["A1"] = f64("me_eq_w") @ f64("me_wv") @ f64("me_wo")
    F["a1"] = (f64("me_eq_b") @ f64("me_wv") + f64("me_bv")) @ f64("me_wo") \
        + f64("me_bo")
    W2o = f64("ce_wv") @ f64("ce_wo")
    F["A2"] = f64("ce_syn_w") + f64("ce_syn_w") @ W2o
    F["a2"] = f64("ce_syn_b") + f64("ce_syn_b") @ W2o + f64("ce_bv") \
        @ f64("ce_wo") + f64("ce_bo")

    wmap = {
        "w1p": (f64("sw_w1"), KC), "w3p": (f64("sw_w3"), KC),
        "w2p": (f64("sw_w2"), KI),
        "c1p": (F["A1"] @ f64("me_c1w"), KC),
        "c2p": (f64("me_c2w"), K2),
        "a2p": (F["A2"], KC), "f1p": (f64("ce_f1w"), KC),
        "f2p": (f64("ce_f2w"), K2), "genp": (f64("ce_gen_w"), KC),
        "m1p": (f64("ml_w1"), KC), "m2p": (f64("ml_w2"), KI),
    }
    bmap = {
        "c1b": F["a1"] @ f64("me_c1w") + f64("me_c1b"),
        "c2b": f64("me_c2b"),
        "a2b": F["a2"],
        "f1b": f64("ce_f1b"), "f2b": f64("ce_f2b"),
        "n1g": f64("ce_n1g"), "n1b": f64("ce_n1b"),
        "n2g": f64("ce_n2g"), "n2b": f64("ce_n2b"),
        "genb": f64("ce_gen_b"), "m1b": f64("ml_b1"), "m2b": f64("ml_b2"),
        "zb": np.zeros(H),
    }
    base = {n: _pack_w(w, kcc) for n, (w, kcc) in wmap.items()}
    base["ball"] = np.concatenate([_pack_b(bmap[n]) for n in BIASES], 1)

    meta = {"x": x, "gates": gates, "sel": sel, "F": F,
            "dev_idx": [], "dev_w": [], "ovf": []}
    in_maps = [dict(base) for _ in range(N_CORES)]
    for e in range(E):
        m = sel == e
        tok = np.nonzero(m.any(1))[0]
        we = np.where(m[:, 0][tok], gates[tok, 0], gates[tok, 1])
        dev, ovf = tok[:CAP], tok[CAP:]
        meta["dev_idx"].append(dev)
        meta["dev_w"].append(we[:len(dev)])
        meta["ovf"].append((ovf, we[len(dev):]))
        xfull = np.zeros((CAP, H), np.float32)
        xfull[:len(dev)] = x[dev]
        percore = xfull.reshape(N_CORES, T, H)
        for c in range(N_CORES):
            xc = percore[c].T.reshape(KC, P, T).transpose(1, 0, 2)
            in_maps[c][f"xg{e}"] = np.ascontiguousarray(xc).astype(BF)
    meta["in_maps"] = in_maps
    return meta


def _host_expert(e, xs, inputs, F):
    """Overflow tokens, fp64, replicating the reference formulas."""
    f64 = lambda n: np.asarray(inputs[n], np.float64)
    xs = xs.astype(np.float64)
    if e == 0:
        a = xs @ f64("sw_w1")
        g = a / (1.0 + np.exp(-a)) * (xs @ f64("sw_w3"))
        return g @ f64("sw_w2")
    if e == 1:
        t = xs @ F["A1"] + F["a1"]
        g = _gelu(t @ f64("me_c1w") + f64("me_c1b"))
        return g @ f64("me_c2w") + f64("me_c2b")
    if e == 2:
        t = xs @ F["A2"] + F["a2"]
        h2 = _ln64(t, f64("ce_n1g"), f64("ce_n1b"))
        ff = np.maximum(h2 @ f64("ce_f1w") + f64("ce_f1b"), 0.0) \
            @ f64("ce_f2w") + f64("ce_f2b")
        h2 = _ln64(h2 + ff, f64("ce_n2g"), f64("ce_n2b"))
        return h2 @ f64("ce_gen_w") + f64("ce_gen_b")
    a = _gelu(xs @ f64("ml_w1") + f64("ml_b1"))
    return a @ f64("ml_w2") + f64("ml_b2")


def combine(meta, results, inputs):
    out = np.zeros((B * S, H), np.float32)
    for e in range(E):
        ye = np.concatenate(
            [results[c][f"y{e}"].transpose(2, 1, 0).reshape(T, H)
             for c in range(N_CORES)], 0)
        dev, we = meta["dev_idx"][e], meta["dev_w"][e]
        out[dev] += (we[:, None] * ye[:len(dev)]).astype(np.float32)
        ovf, wo = meta["ovf"][e]
        if len(ovf):
            yh = _host_expert(e, meta["x"][ovf], inputs, meta["F"])
            out[ovf] += (wo[:, None] * yh).astype(np.float32)
    return out.reshape(B, S, H)


def kernel(**inputs):
    nc = _get_program()
    meta = prepare(inputs)
    res = run_cores(nc, meta["in_maps"])
    return combine(meta, [res.results[c] for c in range(N_CORES)], inputs)


# ---- NTFF profiling shim (axon) — used by test.py only ----------------
def _install_ntff_shim():
    import contextlib
    import ctypes
    import sys
    import types

    if "antenv.axon_hooks" in sys.modules:
        return
    lib = ctypes.CDLL("/opt/axon/libaxon_pjrt.so")
    if not hasattr(lib, "axon_start_nrt_profile"):
        return
    lib.axon_start_nrt_profile.argtypes = [ctypes.POINTER(ctypes.c_int64),
                                           ctypes.c_size_t]
    lib.axon_start_nrt_profile.restype = ctypes.c_int64
    lib.axon_stop_nrt_profile.argtypes = [ctypes.c_char_p]
    lib.axon_stop_nrt_profile.restype = ctypes.c_int64

    @contextlib.contextmanager
    def _hook(output_dir, device_ids):
        import jax
        jax.devices()
        if device_ids:
            ids = (ctypes.c_int64 * len(device_ids))(*device_ids)
            rc = lib.axon_start_nrt_profile(ids, len(device_ids))
        else:
            rc = lib.axon_start_nrt_profile(None, 0)
        if rc != 0:
            raise RuntimeError(f"axon_start_nrt_profile rc={rc}")
        try:
            yield
        finally:
            n = lib.axon_stop_nrt_profile(str(output_dir).encode())
            print(f"profile: {n} file(s) written to {output_dir}",
                  file=sys.stderr)

    import antenv
    mod = types.ModuleType("antenv.axon_hooks")
    mod.get_axon_ntff_profile_hook = lambda: _hook
    mod.set_axon_ntff_profile_hook = lambda hk: None
    sys.modules["antenv.axon_hooks"] = mod
    antenv.axon_hooks = mod


# revision 40
# speedup vs baseline: 1.0045x; 1.0045x over previous
"""Trainium2 Bass kernel for nn_MixtureOfExpertsLayer (moe_routing).

Sparse dispatch: top-2 routing is computed on the host (the router is a
tiny [8192,1024]@[1024,4] GEMM); tokens are gathered per expert and
sharded across the 8 cores so each core runs a fixed 512-token slab
through each of the 4 experts — half the dense FLOPs.  The linear
chains inside experts 1/2 are pre-folded on the host
(eq_w@wv@wo and syn_w@(I+wv@wo)), removing another ~11% of matmul work.

Device compute is bf16 (PSUM accumulates fp32).  Weights are pre-packed
on the host into the exact [p, kc, 256-col] tile layout the tensor
engine wants, so every DMA is a fully contiguous 0.5-2MB block.
Activations stay feature-major [128p, chunk, tok]; every matmul has a
512-token moving dim (full PE rate).  Expert outputs come back
feature-major [H, 512] fp32; the host applies the top-2 softmax gates
and scatter-adds into the final output.  Tokens beyond the
4096-per-expert device capacity (a few dozen when routing is balanced)
are computed on the host in fp64.
"""
import math

import numpy as np
import ml_dtypes

import concourse.bass as bass
import concourse.mybir as mybir
import concourse.tile as tile
from concourse import bacc
from concourse.alu_op_type import AluOpType
from concourse.bass_utils import run_bass_kernel_spmd

F32 = mybir.dt.float32
BF16 = mybir.dt.bfloat16
ACT = mybir.ActivationFunctionType
AX = mybir.AxisListType
OP = AluOpType
BF = ml_dtypes.bfloat16

N_CORES = 8
B, S, H, I, E = 4, 2048, 1024, 4096, 4
P = 128
T = 512                   # tokens per expert per core
CAP = N_CORES * T         # device capacity per expert
KC = H // P               # 8
KI = I // P               # 32
K2 = (2 * H) // P         # 16

# packed weight dram tensors: name -> (n_256col_blocks, contraction_chunks)
PACKED_W = {
    "w1p": (I // 256, KC), "w3p": (I // 256, KC), "m1p": (I // 256, KC),
    "w2p": (H // 256, KI), "m2p": (H // 256, KI),
    "c1p": (2 * H // 256, KC), "f1p": (2 * H // 256, KC),
    "c2p": (H // 256, K2), "f2p": (H // 256, K2),
    "a2p": (H // 256, KC), "genp": (H // 256, KC),
}
# biases live in one packed [P, sum] f32 tensor; name -> n_chunks
BIASES = {
    "c1b": K2, "c2b": KC,
    "a2b": KC, "f1b": K2, "f2b": KC,
    "n1g": KC, "n1b": KC, "n2g": KC, "n2b": KC, "genb": KC,
    "m1b": KI, "m2b": KC, "zb": KC,
}
BIAS_OFF = {}
_off = 0
for _n, _c in BIASES.items():
    BIAS_OFF[_n] = _off
    _off += _c
BIAS_COLS = _off


def build_moe_sparse():
    nc = bacc.Bacc("TRN2", target_bir_lowering=False, debug=False)

    xg = [nc.dram_tensor(f"xg{e}", [P, KC, T], BF16, kind="ExternalInput")
          for e in range(E)]
    wd = {n: nc.dram_tensor(n, [nb, P, kcc, 256], BF16, kind="ExternalInput")
          for n, (nb, kcc) in PACKED_W.items()}
    ball = nc.dram_tensor("ball", [P, BIAS_COLS], F32, kind="ExternalInput")
    ys = [nc.dram_tensor(f"y{e}", [P, KC, T], F32, kind="ExternalOutput")
          for e in range(E)]

    with tile.TileContext(nc) as tc:
        with (
            tc.tile_pool(name="const", bufs=1) as cpool,
            tc.tile_pool(name="xg", bufs=2) as xpool,
            tc.tile_pool(name="h1", bufs=1) as hpool,
            tc.tile_pool(name="inter", bufs=1) as ipool,
            tc.tile_pool(name="ws", bufs=5) as wsp,     # KC-contraction blocks
            tc.tile_pool(name="ws2", bufs=3) as wsp2,   # K2-contraction blocks
            tc.tile_pool(name="wb", bufs=2) as wbp,     # KI-contraction blocks
            tc.tile_pool(name="yev", bufs=3) as ypool,  # output eviction
            tc.tile_pool(name="lns", bufs=1) as lnsp,
            tc.tile_pool(name="lnt", bufs=2) as lntp,
            tc.tile_pool(name="sq", bufs=2) as sqp,
            tc.tile_pool(name="ps", bufs=4, space=bass.MemorySpace.PSUM) as psp,
            tc.tile_pool(name="pss", bufs=2, space=bass.MemorySpace.PSUM) as pssp,
            tc.tile_pool(name="psb", bufs=2, space=bass.MemorySpace.PSUM) as psbp,
        ):
            # ---- constants ------------------------------------------------
            ones_cf = cpool.tile([P, 1], F32, tag="ones_cf")
            nc.vector.memset(ones_cf[:], 1.0)
            ones_c = cpool.tile([P, 1], BF16, tag="ones_c")
            nc.vector.tensor_copy(ones_c[:], ones_cf[:])
            ones_rf = cpool.tile([1, P], F32, tag="ones_rf")
            nc.vector.memset(ones_rf[:], 1.0)
            ones_r = cpool.tile([1, P], BF16, tag="ones_r")
            nc.vector.tensor_copy(ones_r[:], ones_rf[:])

            bt_all = cpool.tile([P, BIAS_COLS], F32, tag="ball")
            bt = {n: bt_all[:, BIAS_OFF[n]:BIAS_OFF[n] + nch]
                  for n, nch in BIASES.items()}

            def load_xg(e, split=1):
                t_ = xpool.tile([P, KC, T], BF16, tag="xg", name=f"xgt{e}")
                step = KC // split
                for i in range(split):
                    sl = slice(i * step, (i + 1) * step)
                    nc.sync.dma_start(t_[:, sl, :], xg[e].ap()[:, sl, :])
                return t_

            h1 = hpool.tile([P, KI, T], BF16, tag="h1")

            # ---- helpers --------------------------------------------------
            def up_proj(dst, wname, src, src_kc, act, bias, blocks=None,
                        pool=None):
                """dst[:, c, :] = act(Wc.T @ src + bias_c), streamed in
                256-col blocks.  dst chunk c = 2*b + ml."""
                pool = pool or wsp
                nb = PACKED_W[wname][0]
                for b_ in (range(nb) if blocks is None else blocks):
                    wc = pool.tile([P, src_kc, 256], BF16, tag="w")
                    nc.sync.dma_start(wc[:], wd[wname].ap()[b_])
                    for ml in range(2):
                        c = 2 * b_ + ml
                        ps = psp.tile([P, T], F32, tag="mm")
                        for kc in range(src_kc):
                            nc.tensor.matmul(
                                ps[:], wc[:, kc, ml * P:(ml + 1) * P],
                                src[:, kc, :],
                                start=(kc == 0), stop=(kc == src_kc - 1))
                        b_sl = None if bias is None else bias[:, c:c + 1]
                        nc.scalar.activation(dst[:, c, :], ps[:], act,
                                             bias=b_sl)

            def out_proj(ydram, wname, src, src_kc, bias, wpool):
                """y[:, c, :] = Wc.T @ src + bias_c -> DMA to DRAM (fp32)."""
                nb = PACKED_W[wname][0]
                for b_ in range(nb):
                    wc = wpool.tile([P, src_kc, 256], BF16, tag="w")
                    nc.sync.dma_start(wc[:], wd[wname].ap()[b_])
                    for ml in range(2):
                        c = 2 * b_ + ml
                        ps = psp.tile([P, T], F32, tag="mm")
                        for kc in range(src_kc):
                            nc.tensor.matmul(
                                ps[:], wc[:, kc, ml * P:(ml + 1) * P],
                                src[:, kc, :],
                                start=(kc == 0), stop=(kc == src_kc - 1))
                        yt = ypool.tile([P, T], F32, tag="y")
                        nc.vector.tensor_scalar(yt[:], ps[:],
                                                bias[:, c:c + 1], None, OP.add)
                        nc.sync.dma_start(ydram.ap()[:, c, :], yt[:])

            def ln_stats(src, tag):
                """Mean/rstd rows (bf16 [1,T]) of feature-major src."""
                ssum = pssp.tile([1, T], F32, tag="st")
                for kc in range(KC):
                    nc.tensor.matmul(ssum[:], ones_c[:], src[:, kc, :],
                                     start=(kc == 0), stop=(kc == KC - 1))
                ssq = pssp.tile([1, T], F32, tag="st")
                for kc in range(KC):
                    sqc = sqp.tile([P, T], BF16, tag="sq")
                    nc.vector.tensor_tensor(sqc[:], src[:, kc, :],
                                            src[:, kc, :], OP.mult)
                    nc.tensor.matmul(ssq[:], ones_c[:], sqc[:],
                                     start=(kc == 0), stop=(kc == KC - 1))
                mu = lnsp.tile([1, T], F32, tag="mu")
                nc.vector.tensor_scalar(mu[:], ssum[:], 1.0 / H, None, OP.mult)
                msq = lnsp.tile([1, T], F32, tag="ms")
                nc.vector.tensor_scalar(msq[:], ssq[:], 1.0 / H, None, OP.mult)
                mu_b = lnsp.tile([1, T], BF16, tag=tag + "mb")
                nc.vector.tensor_copy(mu_b[:], mu[:])
                mu2 = lnsp.tile([1, T], F32, tag="m2")
                nc.vector.tensor_tensor(mu2[:], mu[:], mu[:], OP.mult)
                var = lnsp.tile([1, T], F32, tag="va")
                nc.vector.scalar_tensor_tensor(var[:], msq[:], 1e-5, mu2[:],
                                               OP.add, OP.subtract)
                sdev = lnsp.tile([1, T], F32, tag="sd")
                nc.scalar.activation(sdev[:], var[:], ACT.Sqrt)
                rstd_f = lnsp.tile([1, T], F32, tag="rf")
                nc.vector.reciprocal(rstd_f[:], sdev[:])
                rs_b = lnsp.tile([1, T], BF16, tag=tag + "rb")
                nc.vector.tensor_copy(rs_b[:], rstd_f[:])
                return mu_b, rs_b

            def ln_bcast(mu_b, rs_b):
                """Broadcast [1,T] mean/rstd rows to [P,T] via K=1 matmul."""
                mub = psbp.tile([P, T], F32, tag="bc")
                nc.tensor.matmul(mub[:], ones_r[:], mu_b[:], start=True,
                                 stop=True)
                rsb = psbp.tile([P, T], F32, tag="bc")
                nc.tensor.matmul(rsb[:], ones_r[:], rs_b[:], start=True,
                                 stop=True)
                return mub, rsb

            def ln_norm(dst, src, mub, rsb, g_t, b_t):
                """dst = (src - mu) * rstd * g + b  (bf16 out, DVE only)."""
                for kc in range(KC):
                    t1_ = lntp.tile([P, T], F32, tag="lnt")
                    nc.vector.tensor_tensor(t1_[:], src[:, kc, :], mub[:],
                                            OP.subtract)
                    nc.vector.tensor_tensor(t1_[:], t1_[:], rsb[:], OP.mult)
                    nc.vector.tensor_scalar(dst[:, kc, :], t1_[:],
                                            g_t[:, kc:kc + 1],
                                            b_t[:, kc:kc + 1],
                                            OP.mult, OP.add)

            def ln_apply(dst, src, mu_b, rs_b, g_t, b_t):
                mub, rsb = ln_bcast(mu_b, rs_b)
                ln_norm(dst, src, mub, rsb, g_t, b_t)

            # ---- expert 0: SwiGLU ----------------------------------------
            xt0 = load_xg(0, split=2)
            for b_ in range(I // 256):
                wa = wsp.tile([P, KC, 256], BF16, tag="w")
                nc.sync.dma_start(wa[:], wd["w1p"].ap()[b_])
                wb = wsp.tile([P, KC, 256], BF16, tag="w")
                nc.sync.dma_start(wb[:], wd["w3p"].ap()[b_])
                # defer non-critical loads so startup DMA bandwidth goes
                # to xg0 + the first SwiGLU weight blocks
                if b_ == 4:
                    xt2 = load_xg(2)
                if b_ == 6:
                    nc.sync.dma_start(bt_all[:], ball.ap())
                for ml in range(2):
                    c = 2 * b_ + ml
                    psa = psp.tile([P, T], F32, tag="mm")
                    psb = psp.tile([P, T], F32, tag="mm")
                    for kc in range(KC):
                        nc.tensor.matmul(psa[:], wa[:, kc, ml * P:(ml + 1) * P],
                                         xt0[:, kc, :],
                                         start=(kc == 0), stop=(kc == KC - 1))
                    for kc in range(KC):
                        nc.tensor.matmul(psb[:], wb[:, kc, ml * P:(ml + 1) * P],
                                         xt0[:, kc, :],
                                         start=(kc == 0), stop=(kc == KC - 1))
                    sa = ypool.tile([P, T], F32, tag="sa")
                    nc.scalar.activation(sa[:], psa[:], ACT.Silu)
                    nc.vector.tensor_tensor(h1[:, c, :], psb[:], sa[:],
                                            OP.mult)
            out_proj(ys[0], "w2p", h1, KI, bt["zb"], wbp)

            # ---- expert 2 (part 1): folded front + LN1 stats -------------
            t2 = ipool.tile([P, KC, T], BF16, tag="tA", name="t2")
            up_proj(t2, "a2p", xt2, KC, ACT.Identity, bt["a2b"])
            mu1, rs1 = ln_stats(t2, "l1")

            # ---- expert 1 (filler for LN1 latency); c1p holds A1@C1 ------
            xt1 = load_xg(1)
            g1 = ipool.tile([P, K2, T], BF16, tag="tD", name="g1")
            up_proj(g1, "c1p", xt1, KC, ACT.Gelu, bt["c1b"])
            h2 = ipool.tile([P, KC, T], BF16, tag="tC", name="h2")
            ln_apply(h2, t2, mu1, rs1, bt["n1g"], bt["n1b"])
            out_proj(ys[1], "c2p", g1, K2, bt["c2b"], wsp2)

            # ---- expert 2 (part 2): FF + residual + LN2 stats ------------
            g2 = ipool.tile([P, K2, T], BF16, tag="tD", name="g2")
            up_proj(g2, "f1p", h2, KC, ACT.Relu, bt["f1b"])
            ffa = ipool.tile([P, KC, T], BF16, tag="tB", name="ffa")
            nb_f2 = PACKED_W["f2p"][0]
            for b_ in range(nb_f2):
                wc = wsp2.tile([P, K2, 256], BF16, tag="w")
                nc.sync.dma_start(wc[:], wd["f2p"].ap()[b_])
                for ml in range(2):
                    c = 2 * b_ + ml
                    ps = psp.tile([P, T], F32, tag="mm")
                    for kc in range(K2):
                        nc.tensor.matmul(ps[:], wc[:, kc, ml * P:(ml + 1) * P],
                                         g2[:, kc, :],
                                         start=(kc == 0), stop=(kc == K2 - 1))
                    # ffa = ff + f2b + h2   (residual)
                    nc.vector.scalar_tensor_tensor(
                        ffa[:, c, :], ps[:], bt["f2b"][:, c:c + 1],
                        h2[:, c, :], OP.add, OP.add)
            # ---- expert 3 up-proj interleaved with LN2 + generator -------
            # E3 blocks fill the PE while the DVE drains ffa evictions,
            # computes LN2 stats rows, and normalizes h2b.
            xt3 = load_xg(3)
            up_proj(h1, "m1p", xt3, KC, ACT.Gelu, bt["m1b"],
                    blocks=range(0, 2))
            mu2, rs2 = ln_stats(ffa, "l2")
            up_proj(h1, "m1p", xt3, KC, ACT.Gelu, bt["m1b"],
                    blocks=range(2, 8))
            mub2, rsb2 = ln_bcast(mu2, rs2)
            up_proj(h1, "m1p", xt3, KC, ACT.Gelu, bt["m1b"],
                    blocks=range(8, 16))
            h2b = ipool.tile([P, KC, T], BF16, tag="tA", name="h2b")
            ln_norm(h2b, ffa, mub2, rsb2, bt["n2g"], bt["n2b"])
            out_proj(ys[2], "genp", h2b, KC, bt["genb"], wsp)

            # ---- expert 3 down-projection --------------------------------
            out_proj(ys[3], "m2p", h1, KI, bt["m2b"], wbp)

    nc.compile()
    return nc


_PROGRAM = None


def _get_program():
    global _PROGRAM
    if _PROGRAM is None:
        _PROGRAM = build_moe_sparse()
    return _PROGRAM


def run_cores(nc, in_maps, trace=False, trace_cores=None):
    if trace:
        _install_ntff_shim()
    return run_bass_kernel_spmd(nc, in_maps, core_ids=list(range(len(in_maps))),
                                trace=trace, trace_cores=trace_cores)


# ---- host side ---------------------------------------------------------
def _gelu(x):
    try:
        from scipy.special import erf
        return 0.5 * x * (1.0 + erf(x / math.sqrt(2.0)))
    except ImportError:
        ve = np.vectorize(math.erf)
        return 0.5 * x * (1.0 + ve(x / math.sqrt(2.0)))


def _ln64(h, g, b, eps=1e-5):
    mu = h.mean(-1, keepdims=True)
    var = ((h - mu) ** 2).mean(-1, keepdims=True)
    return (h - mu) / np.sqrt(var + eps) * g + b


def _pack_w(w, kcc):
    """[K, M] fp64 -> [M//256, P, kcc, 256] bf16 contiguous tile blocks."""
    K, M = w.shape
    assert K == kcc * P
    r = w.reshape(kcc, P, M)
    blocks = [np.ascontiguousarray(r[:, :, b * 256:(b + 1) * 256]
                                   .transpose(1, 0, 2))
              for b in range(M // 256)]
    return np.stack(blocks, 0).astype(BF)


def _pack_b(b):
    n = b.shape[0] // P
    return np.ascontiguousarray(b.reshape(n, P).T.astype(np.float32))


def prepare(inputs):
    f64 = lambda n: np.asarray(inputs[n], np.float64)
    x = np.asarray(inputs["x"], np.float32).reshape(-1, H)

    # routing (host, fp64)
    lg = x.astype(np.float64) @ f64("router_w")
    lg += f64("router_b") + f64("load_balancer")
    sel = np.argsort(-lg, axis=1, kind="stable")[:, :2]
    ls = np.take_along_axis(lg, sel, 1)
    ew = np.exp(ls - ls.max(1, keepdims=True))
    gates = ew / ew.sum(1, keepdims=True)

    # folded weights (fp64)
    F = {}
    F["A1"] = f64("me_eq_w") @ f64("me_wv") @ f64("me_wo")
    F["a1"] = (f64("me_eq_b") @ f64("me_wv") + f64("me_bv")) @ f64("me_wo") \
        + f64("me_bo")
    W2o = f64("ce_wv") @ f64("ce_wo")
    F["A2"] = f64("ce_syn_w") + f64("ce_syn_w") @ W2o
    F["a2"] = f64("ce_syn_b") + f64("ce_syn_b") @ W2o + f64("ce_bv") \
        @ f64("ce_wo") + f64("ce_bo")

    wmap = {
        "w1p": (f64("sw_w1"), KC), "w3p": (f64("sw_w3"), KC),
        "w2p": (f64("sw_w2"), KI),
        "c1p": (F["A1"] @ f64("me_c1w"), KC),
        "c2p": (f64("me_c2w"), K2),
        "a2p": (F["A2"], KC), "f1p": (f64("ce_f1w"), KC),
        "f2p": (f64("ce_f2w"), K2), "genp": (f64("ce_gen_w"), KC),
        "m1p": (f64("ml_w1"), KC), "m2p": (f64("ml_w2"), KI),
    }
    bmap = {
        "c1b": F["a1"] @ f64("me_c1w") + f64("me_c1b"),
        "c2b": f64("me_c2b"),
        "a2b": F["a2"],
        "f1b": f64("ce_f1b"), "f2b": f64("ce_f2b"),
        "n1g": f64("ce_n1g"), "n1b": f64("ce_n1b"),
        "n2g": f64("ce_n2g"), "n2b": f64("ce_n2b"),
        "genb": f64("ce_gen_b"), "m1b": f64("ml_b1"), "m2b": f64("ml_b2"),
        "zb": np.zeros(H),
    }
    base = {n: _pack_w(w, kcc) for n, (w, kcc) in wmap.items()}
    base["ball"] = np.concatenate([_pack_b(bmap[n]) for n in BIASES], 1)

    meta = {"x": x, "gates": gates, "sel": sel, "F": F,
            "dev_idx": [], "dev_w": [], "ovf": []}
    in_maps = [dict(base) for _ in range(N_CORES)]
    for e in range(E):
        m = sel == e
        tok = np.nonzero(m.any(1))[0]
        we = np.where(m[:, 0][tok], gates[tok, 0], gates[tok, 1])
        dev, ovf = tok[:CAP], tok[CAP:]
        meta["dev_idx"].append(dev)
        meta["dev_w"].append(we[:len(dev)])
        meta["ovf"].append((ovf, we[len(dev):]))
        xfull = np.zeros((CAP, H), np.float32)
        xfull[:len(dev)] = x[dev]
        percore = xfull.reshape(N_CORES, T, H)
        for c in range(N_CORES):
            xc = percore[c].T.reshape(KC, P, T).transpose(1, 0, 2)
            in_maps[c][f"xg{e}"] = np.ascontiguousarray(xc).astype(BF)
    meta["in_maps"] = in_maps
    return meta


def _host_expert(e, xs, inputs, F):
    """Overflow tokens, fp64, replicating the reference formulas."""
    f64 = lambda n: np.asarray(inputs[n], np.float64)
    xs = xs.astype(np.float64)
    if e == 0:
        a = xs @ f64("sw_w1")
        g = a / (1.0 + np.exp(-a)) * (xs @ f64("sw_w3"))
        return g @ f64("sw_w2")
    if e == 1:
        t = xs @ F["A1"] + F["a1"]
        g = _gelu(t @ f64("me_c1w") + f64("me_c1b"))
        return g @ f64("me_c2w") + f64("me_c2b")
    if e == 2:
        t = xs @ F["A2"] + F["a2"]
        h2 = _ln64(t, f64("ce_n1g"), f64("ce_n1b"))
        ff = np.maximum(h2 @ f64("ce_f1w") + f64("ce_f1b"), 0.0) \
            @ f64("ce_f2w") + f64("ce_f2b")
        h2 = _ln64(h2 + ff, f64("ce_n2g"), f64("ce_n2b"))
        return h2 @ f64("ce_gen_w") + f64("ce_gen_b")
    a = _gelu(xs @ f64("ml_w1") + f64("ml_b1"))
    return a @ f64("ml_w2") + f64("ml_b2")


def combine(meta, results, inputs):
    out = np.zeros((B * S, H), np.float32)
    for e in range(E):
        ye = np.concatenate(
            [results[c][f"y{e}"].transpose(2, 1, 0).reshape(T, H)
             for c in range(N_CORES)], 0)
        dev, we = meta["dev_idx"][e], meta["dev_w"][e]
        out[dev] += (we[:, None] * ye[:len(dev)]).astype(np.float32)
        ovf, wo = meta["ovf"][e]
        if len(ovf):
            yh = _host_expert(e, meta["x"][ovf], inputs, meta["F"])
            out[ovf] += (wo[:, None] * yh).astype(np.float32)
    return out.reshape(B, S, H)


def kernel(**inputs):
    nc = _get_program()
    meta = prepare(inputs)
    # transient NRT/axon device errors (UNAVAILABLE / INTERNAL) have been
    # observed on this fleet and clear on re-run: retry a bounded number
    # of times rather than failing the whole call
    last = None
    for _ in range(3):
        try:
            res = run_cores(nc, meta["in_maps"])
            break
        except Exception as e:
            last = e
    else:
        raise last
    return combine(meta, [res.results[c] for c in range(N_CORES)], inputs)


# ---- NTFF profiling shim (axon) — used by test.py only ----------------
def _install_ntff_shim():
    import contextlib
    import ctypes
    import sys
    import types

    if "antenv.axon_hooks" in sys.modules:
        return
    lib = ctypes.CDLL("/opt/axon/libaxon_pjrt.so")
    if not hasattr(lib, "axon_start_nrt_profile"):
        return
    lib.axon_start_nrt_profile.argtypes = [ctypes.POINTER(ctypes.c_int64),
                                           ctypes.c_size_t]
    lib.axon_start_nrt_profile.restype = ctypes.c_int64
    lib.axon_stop_nrt_profile.argtypes = [ctypes.c_char_p]
    lib.axon_stop_nrt_profile.restype = ctypes.c_int64

    @contextlib.contextmanager
    def _hook(output_dir, device_ids):
        import jax
        jax.devices()
        if device_ids:
            ids = (ctypes.c_int64 * len(device_ids))(*device_ids)
            rc = lib.axon_start_nrt_profile(ids, len(device_ids))
        else:
            rc = lib.axon_start_nrt_profile(None, 0)
        if rc != 0:
            raise RuntimeError(f"axon_start_nrt_profile rc={rc}")
        try:
            yield
        finally:
            n = lib.axon_stop_nrt_profile(str(output_dir).encode())
            print(f"profile: {n} file(s) written to {output_dir}",
                  file=sys.stderr)

    import antenv
    mod = types.ModuleType("antenv.axon_hooks")
    mod.get_axon_ntff_profile_hook = lambda: _hook
    mod.set_axon_ntff_profile_hook = lambda hk: None
    sys.modules["antenv.axon_hooks"] = mod
    antenv.axon_hooks = mod
